# revision 1
# baseline (speedup 1.0000x reference)
"""Causal self-attention (B=2, T=2048, C=1024, H=16) on 8 Trainium2 cores.

Sharding: tensor-parallel over heads (2 heads/core). Each core computes
QKV projection for its heads, causal attention, and a partial c_proj
output; partials are summed on the host (b_proj is added by core 0 only).

Per-core dataflow (everything kept "K-major" so no activation transposes
are needed on the critical path):
  xT [C, B*T]  (host pre-transposes x)
  qT/kT/vT [128, B*T] = W_local^T @ x + b      (PE, fp32r)
  S^T tile [k 128, q 512] = K @ Q^T            (PE)  -- causal tiles only
  E^T = exp(S^T/8) * causal_mask               (ACT + DVE)
  y'^T [65, q 512] += [v | 1]^T @ E^T          (PE; row 64 = softmax sums)
  y_norm^T = y'^T[0:64] * bcast(1/sums)        (PE rank-1 bcast + DVE)
  partial^T [c 128, row 512] = Wp_local^T-ish  (PE) + b_proj  -> DRAM

fp32r is used for all matmuls (full PE rate at free-dim >= 256, ~1e-4
rel err vs fp32). The BIR verifier requires fp32r operands to come from
fp32r-typed producers: DRAM inputs are declared fp32r (same bits as
fp32) so plain HWDGE DMAs satisfy it; intermediates are written as
fp32r by ACT/DVE ops.
"""

import numpy as np

import concourse.bass as bass
import concourse.tile as tile
from concourse import bacc, mybir
from concourse.bass_utils import run_bass_kernel_spmd
from concourse.masks import make_identity

F32 = mybir.dt.float32
F32R = mybir.dt.float32r

B, T, C, H = 2, 2048, 1024, 16
HS = C // H            # 64 head dim
NCORES = 8
HL = H // NCORES       # 2 local heads
LC = HL * HS           # 128 local q/k/v cols
R = B * T              # 4096 rows (b, t)
KC = C // 128          # 8 contraction chunks for projections
QT = 512               # attention q tile (free dim)
NQT = T // QT          # 4
KA = 128               # attention k chunk (partition dim)
NKA = T // KA          # 16
RT = 512               # row tile for projections
NRT = R // RT          # 8
NCC = C // 128         # 8 c_proj output chunks


def build_program():
    nc = bacc.Bacc("TRN2", target_bir_lowering=False, debug=False,
                   num_devices=NCORES)

    xT = nc.dram_tensor("xT", [C, R], F32R, kind="ExternalInput").ap()
    wqkv = nc.dram_tensor("wqkv", [C, 3 * LC], F32R, kind="ExternalInput").ap()
    bqkv = nc.dram_tensor("bqkv", [3 * LC], F32, kind="ExternalInput").ap()
    wp = nc.dram_tensor("wp", [LC, C], F32R, kind="ExternalInput").ap()
    bp = nc.dram_tensor("bp", [C], F32, kind="ExternalInput").ap()
    trimask = nc.dram_tensor("trimask", [KA, KA], F32R, kind="ExternalInput").ap()
    outT = nc.dram_tensor("outT", [C, R], F32, kind="ExternalOutput").ap()

    with tile.TileContext(nc) as tc:
        with (
            tc.tile_pool(name="consts", bufs=1) as consts,
            tc.tile_pool(name="weights", bufs=1) as weights,
            tc.tile_pool(name="qkvT", bufs=1) as qkvT_pool,
            tc.tile_pool(name="xs", bufs=3) as xs_pool,
            tc.tile_pool(name="vp", bufs=2 * NKA) as vp_pool,
            tc.tile_pool(name="et", bufs=10) as et_pool,
            tc.tile_pool(name="ysb", bufs=3) as ysb_pool,
            tc.tile_pool(name="rec", bufs=2) as rec_pool,
            tc.tile_pool(name="osb", bufs=8) as osb_pool,
            tc.tile_pool(name="dscr", bufs=4, space="DRAM") as dscr_pool,
            tc.tile_pool(name="mm512", bufs=3, space="PSUM") as mm512_pool,
            tc.tile_pool(name="ytps", bufs=2, space="PSUM") as ytps_pool,
            tc.tile_pool(name="smps", bufs=3, space="PSUM") as smps_pool,
        ):
            # ---- constants ----
            identity = consts.tile([128, 128], F32)
            make_identity(nc, identity)
            ones64_f = consts.tile([1, HS], F32)
            nc.vector.memset(ones64_f, 1.0)
            ones64 = consts.tile([1, HS], F32R)
            nc.vector.tensor_copy(ones64, ones64_f)
            ones_col = consts.tile([128, 1], F32)
            nc.vector.memset(ones_col, 1.0)
            tri_sb = consts.tile([KA, KA], F32R)
            bqkv_sb = consts.tile([128, 3], F32)
            bp_sb = consts.tile([128, NCC], F32)

            # ---- weights (fp32r-typed DRAM, plain HWDGE loads) ----
            wq_sb = weights.tile([128, KC, 3 * LC], F32R)
            wq_r = wqkv.rearrange("(kc p) n -> p kc n", p=128)
            nc.sync.dma_start(out=wq_sb[:, 0:2], in_=wq_r[:, 0:2])
            nc.sync.dma_start(
                out=bqkv_sb, in_=bqkv.rearrange("(j p) -> p j", p=128))
            wp_sb = weights.tile([LC, C], F32R)

            def load_consts():
                nc.sync.dma_start(out=tri_sb, in_=trimask)
                nc.sync.dma_start(
                    out=bp_sb, in_=bp.rearrange("(j p) -> p j", p=128))
                nc.sync.dma_start(out=wp_sb, in_=wp)

            # ---- phase 1: QKV projection (transposed outputs) ----
            qT_s = qkvT_pool.tile([LC, R], F32R, tag="qT")
            kT_s = qkvT_pool.tile([LC, R], F32R, tag="kT")
            vT_s = qkvT_pool.tile([LC, R], F32R, tag="vT")
            dst_tiles = [qT_s, kT_s, vT_s]

            def qkv_load(rt):
                x_sb = xs_pool.tile([128, KC, RT], F32R, tag="xs", name=f"x_sb_rt{rt}")
                x_r = xT[:, rt * RT:(rt + 1) * RT].rearrange(
                    "(kc p) r -> p kc r", p=128)
                if rt == 0:
                    for kc in range(0, KC, 2):
                        nc.scalar.dma_start(out=x_sb[:, kc:kc + 2],
                                            in_=x_r[:, kc:kc + 2])
                else:
                    nc.scalar.dma_start(out=x_sb[:, 0:KC // 2],
                                        in_=x_r[:, 0:KC // 2])
                    nc.scalar.dma_start(out=x_sb[:, KC // 2:],
                                        in_=x_r[:, KC // 2:])
                return x_sb

            def qkv_compute(rt, x_sb):
                if rt == 0:
                    # kc-outer for the very first tile: matmuls start as soon
                    # as the first x/w chunk lands instead of after all 8
                    pss = [mm512_pool.tile([128, RT], F32, tag="mm512",
                                           name=f"qkv_ps_rt0c{col}")
                           for col in range(3)]
                    for kc in range(KC):
                        for col in range(3):
                            nc.tensor.matmul(
                                pss[col],
                                wq_sb[:, kc, col * LC:(col + 1) * LC],
                                x_sb[:, kc, :],
                                start=(kc == 0),
                                stop=(kc == KC - 1),
                            )
                    for col in range(3):
                        nc.vector.tensor_scalar_add(
                            dst_tiles[col][:, 0:RT],
                            pss[col],
                            bqkv_sb[:, col:col + 1],
                        )
                    return
                for col in range(3):
                    ps = mm512_pool.tile([128, RT], F32, tag="mm512",
                                         name=f"qkv_ps_rt{rt}c{col}")
                    for kc in range(KC):
                        nc.tensor.matmul(
                            ps,
                            wq_sb[:, kc, col * LC:(col + 1) * LC],
                            x_sb[:, kc, :],
                            start=(kc == 0),
                            stop=(kc == KC - 1),
                        )
                    # PSUM -> SBUF with per-partition bias add, rounding to f32r
                    nc.vector.tensor_scalar_add(
                        dst_tiles[col][:, rt * RT:(rt + 1) * RT],
                        ps,
                        bqkv_sb[:, col:col + 1],
                    )

            def proj_rowtile(rt, tail):
                """c_proj partial for row tile rt (needs ynT rows complete)."""
                half = RT // 2
                for cc in range(NCC):
                    if tail and cc % 2 == 1:
                        pps = smps_pool.tile([128, RT], F32, tag="sm",
                                             name=f"pps_rt{rt}c{cc}")
                    else:
                        pps = mm512_pool.tile([128, RT], F32, tag="mm512",
                                              name=f"pps_rt{rt}c{cc}")
                    nc.tensor.matmul(
                        pps,
                        wp_sb[:, cc * 128:(cc + 1) * 128],
                        ynT_s[:, rt * RT:(rt + 1) * RT],
                        start=True,
                        stop=True,
                    )
                    o_sb = osb_pool.tile([128, RT], F32, tag="osb",
                                         name=f"o_sb_rt{rt}c{cc}")
                    if tail:
                        # both engines are idle in the tail: split for slot
                        # turnover
                        nc.scalar.activation(
                            o_sb[:, 0:half], pps[:, 0:half],
                            mybir.ActivationFunctionType.Identity,
                            bias=bp_sb[:, cc:cc + 1],
                        )
                        nc.vector.tensor_scalar_add(
                            o_sb[:, half:RT], pps[:, half:RT],
                            bp_sb[:, cc:cc + 1])
                    else:
                        # overlapped with exp-bound attention: keep ACT free
                        nc.vector.tensor_scalar_add(o_sb, pps,
                                                    bp_sb[:, cc:cc + 1])
                    nc.sync.dma_start(
                        out=outT[cc * 128:(cc + 1) * 128,
                                 rt * RT:(rt + 1) * RT],
                        in_=o_sb,
                    )

            # ---- phase 2: attention per (b, h), interleaved with QKV/proj ----
            ynT_s = qkvT_pool.tile([LC, R], F32R, tag="ynT")
            x0 = qkv_load(0)
            nc.sync.dma_start(out=wq_sb[:, 2:4], in_=wq_r[:, 2:4])
            nc.sync.dma_start(out=wq_sb[:, 4:KC], in_=wq_r[:, 4:KC])
            qkv_compute(0, x0)
            x1 = qkv_load(1)
            load_consts()
            qkv_compute(1, x1)
            for rt in range(2, NRT // 2):
                qkv_compute(rt, qkv_load(rt))
            for b in range(B):
                base = b * T
                if b + 1 < B:
                    for rt in range((b + 1) * NRT // 2, (b + 2) * NRT // 2):
                        qkv_compute(rt, qkv_load(rt))
                for h in range(HL):
                    hsl = slice(h * HS, (h + 1) * HS)
                    vps = []

                    # qt-outer: only one y' accumulator live at a time
                    for qt in range(NQT):
                        # v' tiles [k 128, 64 v-cols | ones] for the k chunks
                        # this qt introduces -- lazy prep keeps (h, qt)
                        # dependent only on QKV row tiles <= qt
                        for kc in range(qt * (QT // KA),
                                        (qt + 1) * (QT // KA)):
                            vp = vp_pool.tile([KA, HS + 1], F32R,
                                              name=f"vp_b{b}h{h}k{kc}",
                                              tag="vp")
                            tp = mm512_pool.tile([KA, HS], F32, tag="mm512",
                                                 name=f"tp_b{b}h{h}k{kc}")
                            nc.tensor.transpose(
                                tp,
                                vT_s[hsl,
                                     base + kc * KA: base + (kc + 1) * KA]
                                .bitcast(F32),
                                identity[hsl, hsl],
                            )
                            nc.vector.tensor_copy(vp[:, 0:HS], tp)
                            nc.gpsimd.tensor_copy(vp[:, HS:HS + 1], ones_col)
                            vps.append(vp)
                        yp = ytps_pool.tile([HS + 1, QT], F32, tag="yt",
                                            name=f"yt_b{b}h{h}q{qt}")
                        nka_q = (qt + 1) * (QT // KA)
                        for kc in range(nka_q):
                            diag = (kc * KA // QT == qt)
                            sps = smps_pool.tile(
                                [KA, QT], F32, tag="sm",
                                name=f"sps_b{b}h{h}q{qt}k{kc}")
                            nc.tensor.matmul(
                                sps,
                                kT_s[hsl,
                                     base + kc * KA: base + (kc + 1) * KA],
                                qT_s[hsl,
                                     base + qt * QT: base + (qt + 1) * QT],
                                start=True,
                                stop=True,
                            )
                            et = et_pool.tile([KA, QT], F32R, tag="et",
                                              name=f"et_b{b}h{h}q{qt}k{kc}")
                            # columns < off of a diagonal tile are fully
                            # masked; skip them entirely (the AV matmul
                            # accumulates only the [off, QT) span).
                            off = kc * KA - qt * QT if diag else 0
                            nc.scalar.activation(
                                et[:, off:QT], sps[:, off:QT],
                                mybir.ActivationFunctionType.Exp,
                                scale=1.0 / np.sqrt(HS).item(),
                            )
                            if diag:
                                # [off, off+128) is the triangular block
                                nc.gpsimd.tensor_mul(
                                    et[:, off:off + KA],
                                    et[:, off:off + KA],
                                    tri_sb,
                                )
                            nc.tensor.matmul(
                                yp[:, off:QT],
                                vps[kc],
                                et[:, off:QT],
                                start=(kc == 0),
                                stop=(kc == nka_q - 1),
                            )

                        # normalize: y_norm^T = y'^T[0:64] * bcast(1 / sums)
                        yts = ysb_pool.tile([HS + 1, QT], F32, tag="yts",
                                            name=f"yts_b{b}h{h}q{qt}")
                        nc.vector.tensor_copy(yts, yp)
                        rec = rec_pool.tile([1, QT], F32R, tag="rec",
                                            name=f"rec_b{b}h{h}q{qt}")
                        with nc.allow_low_precision(
                                reason="fp32r reciprocal: ~1e-4 rel err ok"):
                            nc.vector.reciprocal(rec, yts[HS:HS + 1, :])
                        if qt == NQT - 1 and h == HL - 1:
                            # end of batch: nothing else keeps PE busy, and
                            # the DRAM-bounce latency would gate the final
                            # c_proj row tile -- use a rank-1 PE broadcast
                            bcp = smps_pool.tile([HS, QT], F32, tag="sm",
                                                 name=f"bcp_b{b}h{h}q{qt}")
                            nc.tensor.matmul(bcp, ones64, rec,
                                             start=True, stop=True)
                            nc.vector.tensor_mul(
                                ynT_s[hsl,
                                      base + qt * QT: base + (qt + 1) * QT],
                                yts[0:HS, :],
                                bcp,
                            )
                        else:
                            bcs = ysb_pool.tile([HS, QT], F32R, tag="bcs",
                                                name=f"bcs_b{b}h{h}q{qt}")
                            recd = dscr_pool.tile([1, QT], F32R, tag="recd",
                                                  name=f"recd_b{b}h{h}q{qt}")
                            nc.sync.dma_start(out=recd, in_=rec)
                            rec_bcast = bass.AP(
                                tensor=recd.tensor, offset=recd.offset,
                                ap=[[0, HS]] + [list(d) for d in recd.ap[1:]])
                            nc.sync.dma_start(out=bcs, in_=rec_bcast)
                            nc.vector.tensor_mul(
                                ynT_s[hsl,
                                      base + qt * QT: base + (qt + 1) * QT],
                                yts[0:HS, :],
                                bcs,
                            )
                        # c_proj row tiles interleave into the last head's
                        # attention, one qt behind the normalize that feeds
                        # them, so the PE never waits on the bcast chain and
                        # output DMA spreads across the attention window.
                        if h == HL - 1 and qt > 0:
                            proj_rowtile(b * NRT // 2 + qt - 1,
                                         tail=(b == B - 1 and qt == NQT - 1))

                # last row tile of this batch after its attention finishes
                proj_rowtile(b * NRT // 2 + NQT - 1, tail=(b == B - 1))

    nc.compile()
    return nc


_NC = None


def _get_nc():
    global _NC
    if _NC is None:
        _NC = build_program()
    return _NC


def make_in_maps(x, W_attn, b_attn, W_proj, b_proj):
    x = np.asarray(x, np.float32)
    W_attn = np.asarray(W_attn, np.float32)
    b_attn = np.asarray(b_attn, np.float32)
    W_proj = np.asarray(W_proj, np.float32)
    b_proj = np.asarray(b_proj, np.float32)

    xT = np.ascontiguousarray(x.reshape(R, C).T)
    tri = np.triu(np.ones((KA, KA), np.float32))  # [kk, j]: 1 if j >= kk
    zeros_bp = np.zeros_like(b_proj)

    in_maps = []
    for core in range(NCORES):
        g0 = core * HL * HS  # first local column in head space
        cols = slice(g0, g0 + LC)
        w_local = np.concatenate(
            [W_attn[:, i * C:(i + 1) * C][:, cols] for i in range(3)], axis=1)
        b_local = np.concatenate(
            [b_attn[i * C:(i + 1) * C][cols] for i in range(3)])
        in_maps.append({
            "xT": xT,
            "wqkv": np.ascontiguousarray(w_local),
            "bqkv": np.ascontiguousarray(b_local),
            "wp": np.ascontiguousarray(W_proj[cols, :]),
            "bp": b_proj if core == 0 else zeros_bp,
            "trimask": tri,
        })
    return in_maps


def kernel(x, W_attn, b_attn, W_proj, b_proj):
    nc = _get_nc()
    in_maps = make_in_maps(x, W_attn, b_attn, W_proj, b_proj)
    res = run_bass_kernel_spmd(nc, in_maps, list(range(NCORES)))
    acc = res.results[0]["outT"].copy()
    for corer in res.results[1:]:
        acc += corer["outT"]
    return np.ascontiguousarray(acc.T).reshape(B, T, C)



# revision 4
# speedup vs baseline: 1.0404x; 1.0404x over previous
"""Causal self-attention (B=2, T=2048, C=1024, H=16) on 8 Trainium2 cores.

Sharding: tensor-parallel over heads (2 heads/core). Each core computes
QKV projection for its heads, causal attention, and a partial c_proj
output; partials are summed on the host. The v-projection bias and
b_proj fold into the host reduction (softmax weights sum to 1, so the
v-bias contributes the constant vector b_v @ W_proj to every row).

All matmuls run in bf16 (1 PE cycle/row at any free size under the
cost model; rel err budget is 2e-2 and bf16 lands ~1e-3 end to end).

Per-core dataflow, everything K-major so no PE transposes at all:
  xT [C, B*T] bf16 (host pre-transposes x)
  qT/kT [128, B*T] = Wqk^T @ x + b            (PE; epilogue adds bias)
  v     [r 128, 65]  = x^T-stationary matmul  (PE; direct [r, hs] layout,
                                               ones col for softmax sums)
  S^T pair [k 128, 2*512] = K @ Q^T           (PE; causal tiles only)
  E^T = exp(S^T/8) over the flat [128, <=1024] span  (ACT, 2 tiles/inst)
  diag 128x128 blocks masked post-exp         (Pool, 0/1 trimask)
  y'^T [65, 512] += v_aug^T @ E^T             (PE; row 64 = softmax sums)
  rec = 1/y'[64] (DVE, PSUM direct); bcast via rank-1 PE matmul
  ynT = y'[0:64] * bcast                      (DVE)
  partial^T [c 128, 512] = Wp_local^T @ ynT   (PE) -> bf16 -> DRAM

The PE executes in order, so emission order = PE schedule: background
work (next batch's QKV/v projections, c_proj row tiles) is drip-fed
into the attention kc loop one thunk at a time to cover the exp
latency (ACT is slightly slower per tile pair than PE).
"""

import numpy as np
import ml_dtypes

import concourse.bass as bass
import concourse.tile as tile
from concourse import bacc, mybir
from concourse.bass_utils import run_bass_kernel_spmd

F32 = mybir.dt.float32
F32R = mybir.dt.float32r
BF16 = mybir.dt.bfloat16

B, T, C, H = 2, 2048, 1024, 16
HS = C // H            # 64 head dim
NCORES = 8
HL = H // NCORES       # 2 local heads
LC = HL * HS           # 128 local q/k/v cols
R = B * T              # 4096 rows
KC = C // 128          # 8 contraction chunks for projections
QT = 512               # attention q tile
NQT = T // QT          # 4
KA = 128               # attention k chunk
RT = 512               # row tile for projections
NRT = R // RT          # 8
NCC = C // 128         # 8 c_proj output chunks
BF = ml_dtypes.bfloat16


def _flat(t, lo, hi):
    """Contiguous free-dim span [lo, hi) of a tile viewed as [part, hi-lo]."""
    return bass.AP(tensor=t.tensor, offset=t.offset + lo,
                   ap=[list(t.ap[0]), [1, hi - lo]])


def build_program():
    nc = bacc.Bacc("TRN2", target_bir_lowering=False, debug=False,
                   num_devices=NCORES)

    xT = nc.dram_tensor("xT", [C, R], BF16, kind="ExternalInput").ap()
    wqk = nc.dram_tensor("wqk", [C, 2 * LC], BF16, kind="ExternalInput").ap()
    wv = nc.dram_tensor("wv", [C, LC], BF16, kind="ExternalInput").ap()
    bqk = nc.dram_tensor("bqk", [2 * LC], F32, kind="ExternalInput").ap()
    wp = nc.dram_tensor("wp", [LC, C], BF16, kind="ExternalInput").ap()
    trimask = nc.dram_tensor("trimask", [KA, KA], BF16,
                             kind="ExternalInput").ap()
    outT = nc.dram_tensor("outT", [C, R], BF16, kind="ExternalOutput").ap()

    with tile.TileContext(nc) as tc:
        with (
            tc.tile_pool(name="consts", bufs=1) as consts,
            tc.tile_pool(name="weights", bufs=1) as weights,
            tc.tile_pool(name="qkvT", bufs=1) as qkvT_pool,
            tc.tile_pool(name="xs", bufs=NRT) as xs_pool,
            tc.tile_pool(name="vh", bufs=2 * B * T // KA) as vh_pool,
            tc.tile_pool(name="et", bufs=4) as et_pool,
            tc.tile_pool(name="rec", bufs=2) as rec_pool,
            tc.tile_pool(name="osb", bufs=4) as osb_pool,
            tc.tile_pool(name="mm512", bufs=2, space="PSUM") as mm512_pool,
            tc.tile_pool(name="ytps", bufs=2, space="PSUM") as ytps_pool,
            tc.tile_pool(name="smps", bufs=2, space="PSUM") as smps_pool,
        ):
            # ---- constants ----
            ones64_f = consts.tile([1, HS], F32)
            nc.vector.memset(ones64_f, 1.0)
            ones64 = consts.tile([1, HS], F32R)
            nc.vector.tensor_copy(ones64, ones64_f)
            tri_sb = consts.tile([KA, KA], BF16)
            bqk_sb = consts.tile([128, 2], F32)

            wqk_sb = weights.tile([128, KC, 2 * LC], BF16)
            wv_sb = weights.tile([128, KC, LC], BF16)
            wp_sb = weights.tile([LC, C], BF16)

            wqk_r = wqk.rearrange("(kc p) n -> p kc n", p=128)
            wv_r = wv.rearrange("(kc p) n -> p kc n", p=128)

            # minimal first loads so PE starts fast; weights on SP queue,
            # x on ACT queue (all x is prefetched up front: no DMA waits
            # inside the attention phase)
            nc.sync.dma_start(out=wqk_sb[:, 0:1], in_=wqk_r[:, 0:1])
            nc.sync.dma_start(
                out=bqk_sb, in_=bqk.rearrange("(j p) -> p j", p=128))
            x_tiles = []
            for rt in range(NRT):
                x_sb = xs_pool.tile([128, KC, RT], BF16, tag="xs",
                                    name=f"x_sb{rt}")
                x_tiles.append(x_sb)
            x_r0 = xT[:, 0:RT].rearrange("(kc p) r -> p kc r", p=128)
            nc.scalar.dma_start(out=x_tiles[0][:, 0:1], in_=x_r0[:, 0:1])
            nc.scalar.dma_start(out=x_tiles[0][:, 1:4], in_=x_r0[:, 1:4])
            nc.sync.dma_start(out=wqk_sb[:, 1:4], in_=wqk_r[:, 1:4])
            nc.scalar.dma_start(out=x_tiles[0][:, 4:KC], in_=x_r0[:, 4:KC])
            nc.sync.dma_start(out=wqk_sb[:, 4:KC], in_=wqk_r[:, 4:KC])
            nc.sync.dma_start(out=wv_sb, in_=wv_r)
            nc.sync.dma_start(out=tri_sb, in_=trimask)
            nc.sync.dma_start(out=wp_sb, in_=wp)
            for rt in range(1, NRT):
                x_r = xT[:, rt * RT:(rt + 1) * RT].rearrange(
                    "(kc p) r -> p kc r", p=128)
                nc.scalar.dma_start(out=x_tiles[rt][:, 0:KC // 2],
                                    in_=x_r[:, 0:KC // 2])
                nc.scalar.dma_start(out=x_tiles[rt][:, KC // 2:],
                                    in_=x_r[:, KC // 2:])

            qT_s = qkvT_pool.tile([LC, R], BF16, tag="qT")
            kT_s = qkvT_pool.tile([LC, R], BF16, tag="kT")
            ynT_s = qkvT_pool.tile([LC, R], BF16, tag="ynT")

            vh_tiles = {}   # (b, chunk) -> [128, 130] tile (65 per head)
            epi_rr = [0]    # epilogue engine round-robin state

            def psum_to_sbuf(dst, src, bias=None):
                """PSUM->SBUF epilogue on a rotating engine."""
                i = epi_rr[0]
                epi_rr[0] += 1
                if bias is not None:
                    if i % 2 == 0:
                        nc.vector.tensor_scalar_add(dst, src, bias)
                    else:
                        nc.scalar.activation(
                            dst, src, mybir.ActivationFunctionType.Identity,
                            bias=bias)
                    return
                if i % 2 == 0:
                    nc.vector.tensor_copy(dst, src)
                else:
                    nc.scalar.activation(
                        dst, src, mybir.ActivationFunctionType.Identity)

            def qk_mm(rt, col, lohi, ps):
                dst = qT_s if col == 0 else kT_s
                for kc in range(*lohi):
                    nc.tensor.matmul(
                        ps,
                        wqk_sb[:, kc, col * LC:(col + 1) * LC],
                        x_tiles[rt][:, kc, :],
                        start=(kc == 0),
                        stop=(kc == KC - 1),
                    )
                if lohi[1] == KC:
                    psum_to_sbuf(dst[:, rt * RT:(rt + 1) * RT], ps,
                                 bias=bqk_sb[:, col:col + 1])

            def v_mm(rt, j, ps):
                """v for row chunk (rt*4+j) directly in [r, hs] layout."""
                for kc in range(KC):
                    nc.tensor.matmul(
                        ps[:, j * KA:(j + 1) * KA],
                        x_tiles[rt][:, kc, j * KA:(j + 1) * KA],
                        wv_sb[:, kc, :],
                        start=(kc == 0),
                        stop=(kc == KC - 1),
                    )

            def v_epi(rt, j, ps):
                chunk = rt * (RT // KA) + j
                b = chunk // (T // KA)
                ch = chunk % (T // KA)
                vh = vh_pool.tile([128, 2 * (HS + 1)], BF16, tag="vh",
                                  name=f"vh_b{b}c{ch}")
                src = ps[:, j * KA:(j + 1) * KA]
                nc.vector.tensor_copy(
                    bass.AP(tensor=vh.tensor, offset=vh.offset,
                            ap=[list(vh.ap[0]), [HS + 1, 2], [1, HS]]),
                    bass.AP(tensor=src.tensor, offset=src.offset,
                            ap=[list(src.ap[0]), [HS, 2], [1, HS]]),
                )
                nc.gpsimd.memset(
                    bass.AP(tensor=vh.tensor, offset=vh.offset + HS,
                            ap=[list(vh.ap[0]), [HS + 1, 2], [1, 1]]),
                    1.0)
                vh_tiles[(b, ch)] = vh

            def qkv_rt_thunks(rt):
                """Thunk list for one 512-row QKV tile (~2.5us of PE)."""
                ps_qk = [None, None]
                ps_v = [None]

                def qk_a(col):
                    def f():
                        ps_qk[col] = mm512_pool.tile(
                            [128, RT], F32, tag="mm512",
                            name=f"qkps{rt}c{col}")
                        qk_mm(rt, col, (0, KC // 2), ps_qk[col])
                    return f

                def qk_b(col):
                    return lambda: qk_mm(rt, col, (KC // 2, KC), ps_qk[col])

                def v_a(j):
                    def f():
                        if j == 0:
                            ps_v[0] = mm512_pool.tile(
                                [128, RT], F32, tag="mm512",
                                name=f"vps{rt}")
                        v_mm(rt, j, ps_v[0])
                    return f

                def v_b(j):
                    return lambda: v_epi(rt, j, ps_v[0])

                out = [qk_a(0), qk_b(0), qk_a(1), qk_b(1)]
                for j in range(RT // KA):
                    out.append(v_a(j))
                for j in range(RT // KA):
                    out.append(v_b(j))
                return out

            o2_state = {}

            def proj_thunk(rt, cc):
                """One c_proj output chunk: matmul + epilogue (+store)."""
                def f():
                    pp = mm512_pool.tile([128, RT], F32, tag="mm512",
                                         name=f"pp{rt}c{cc}")
                    nc.tensor.matmul(
                        pp,
                        wp_sb[:, cc * 128:(cc + 1) * 128],
                        ynT_s[:, rt * RT:(rt + 1) * RT],
                        start=True,
                        stop=True,
                    )
                    if cc % 2 == 0:
                        o2_state[rt] = osb_pool.tile(
                            [128, 2, RT], BF16, tag="osb",
                            name=f"o2_{rt}_{cc}")
                    o2 = o2_state[rt]
                    psum_to_sbuf(o2[:, cc % 2], pp)
                    if cc % 2 == 1:
                        dst = outT[(cc - 1) * 128:(cc + 1) * 128,
                                   rt * RT:(rt + 1) * RT]
                        nc.sync.dma_start(
                            out=dst.rearrange("(g p) r -> p g r", p=128),
                            in_=o2)
                return f

            bg = []

            def bg_step(n=1):
                for _ in range(n):
                    if bg:
                        bg.pop(0)()

            # ---- phase A: QKV + v for batch 0 (rows 0..2047) ----
            for rt in range(NRT // 2):
                for th in qkv_rt_thunks(rt):
                    th()

            # ---- attention, batch-by-batch ----
            for b in range(B):
                base = b * T
                if b == 0:
                    for rt in range(NRT // 2, NRT):
                        bg.extend(qkv_rt_thunks(rt))
                for h in range(HL):
                    hsl = slice(h * HS, (h + 1) * HS)
                    for qt in range(NQT):
                        qcols = slice(base + qt * QT, base + (qt + 1) * QT)
                        nka = (qt + 1) * (QT // KA)
                        pairs = [(kc, kc + 1) for kc in range(0, nka, 2)]
                        pend = None

                        def av_pair(info):
                            sps2, et2, p2 = info
                            for i, kc in enumerate(p2):
                                diag = (kc * KA // QT == qt)
                                off = kc * KA - qt * QT if diag else 0
                                nc.tensor.matmul(
                                    yp[:, off:QT],
                                    vh_tiles[(b, kc)][:,
                                                      h * (HS + 1):
                                                      (h + 1) * (HS + 1)],
                                    et2[:, i, off:QT],
                                    start=(kc == 0),
                                    stop=(kc == nka - 1),
                                )

                        yp = ytps_pool.tile([HS + 1, QT], F32, tag="yt",
                                            name=f"yt_b{b}h{h}q{qt}")
                        for p2 in pairs:
                            sps2 = smps_pool.tile(
                                [128, 2, QT], F32, tag="sm",
                                name=f"sps_b{b}h{h}q{qt}k{p2[0]}")
                            for i, kc in enumerate(p2):
                                nc.tensor.matmul(
                                    sps2[:, i],
                                    kT_s[hsl,
                                         base + kc * KA:base + (kc + 1) * KA],
                                    qT_s[hsl, qcols],
                                    start=True,
                                    stop=True,
                                )
                            et2 = et_pool.tile([128, 2, QT], BF16, tag="et",
                                               name=f"et_b{b}h{h}q{qt}k{p2[0]}")
                            d0 = (p2[0] * KA // QT == qt)
                            lo = p2[0] * KA - qt * QT if d0 else 0
                            nc.scalar.activation(
                                _flat(et2, lo, 2 * QT),
                                _flat(sps2, lo, 2 * QT),
                                mybir.ActivationFunctionType.Exp,
                                scale=1.0 / np.sqrt(HS).item(),
                            )
                            for i, kc in enumerate(p2):
                                if kc * KA // QT == qt:
                                    o = kc * KA - qt * QT
                                    nc.gpsimd.tensor_mul(
                                        et2[:, i, o:o + KA],
                                        et2[:, i, o:o + KA],
                                        tri_sb,
                                    )
                            if pend is not None:
                                av_pair(pend)
                            pend = (sps2, et2, p2)
                            bg_step(1)
                        av_pair(pend)

                        # normalize: rec on DVE straight from PSUM, rank-1
                        # PE broadcast, then scale into ynT. The multiply
                        # cannot take two PSUM sources, so y bounces
                        # through SBUF (ACT to keep DVE free for the mul).
                        rec = rec_pool.tile([1, QT], F32R, tag="rec",
                                            name=f"rec_b{b}h{h}q{qt}")
                        with nc.allow_low_precision(
                                reason="f32r reciprocal: ~1e-4 rel err ok"):
                            nc.vector.reciprocal(rec, yp[HS:HS + 1, :])
                        yc = rec_pool.tile([HS, QT], F32, tag="yc",
                                           name=f"yc_b{b}h{h}q{qt}")
                        nc.scalar.activation(
                            yc, yp[0:HS, :],
                            mybir.ActivationFunctionType.Identity)
                        bcp = mm512_pool.tile([HS, QT], F32, tag="mm512",
                                              name=f"bcp_b{b}h{h}q{qt}")
                        nc.tensor.matmul(bcp, ones64, rec,
                                         start=True, stop=True)
                        nc.vector.tensor_mul(
                            ynT_s[hsl, qcols], yc, bcp)
                        bg_step(2)
                        if h == HL - 1:
                            rt = b * (NRT // 2) + qt
                            for cc in range(NCC):
                                bg.append(proj_thunk(rt, cc))
                # drain background before switching batches so batch-1
                # attention starts with a clean queue
                while bg and b == 0:
                    bg_step(1)
            while bg:
                bg_step(1)

    nc.compile()
    return nc


_NC = None


def _get_nc():
    global _NC
    if _NC is None:
        _NC = build_program()
    return _NC


def make_in_maps(x, W_attn, b_attn, W_proj, b_proj):
    x = np.asarray(x, np.float32)
    W_attn = np.asarray(W_attn, np.float32)
    b_attn = np.asarray(b_attn, np.float32)
    W_proj = np.asarray(W_proj, np.float32)

    xT = np.ascontiguousarray(x.reshape(R, C).T).astype(BF)
    tri = np.triu(np.ones((KA, KA), np.float32)).astype(BF)

    in_maps = []
    for core in range(NCORES):
        g0 = core * LC
        cols = slice(g0, g0 + LC)
        wqk_l = np.concatenate(
            [W_attn[:, 0:C][:, cols], W_attn[:, C:2 * C][:, cols]], axis=1)
        bqk_l = np.concatenate(
            [b_attn[0:C][cols], b_attn[C:2 * C][cols]])
        in_maps.append({
            "xT": xT,
            "wqk": np.ascontiguousarray(wqk_l).astype(BF),
            "wv": np.ascontiguousarray(
                W_attn[:, 2 * C:3 * C][:, cols]).astype(BF),
            "bqk": np.ascontiguousarray(bqk_l),
            "wp": np.ascontiguousarray(W_proj[cols, :]).astype(BF),
            "trimask": tri,
        })
    return in_maps


def kernel(x, W_attn, b_attn, W_proj, b_proj):
    nc = _get_nc()
    in_maps = make_in_maps(x, W_attn, b_attn, W_proj, b_proj)
    res = run_bass_kernel_spmd(nc, in_maps, list(range(NCORES)))
    acc = res.results[0]["outT"].astype(np.float32)
    for corer in res.results[1:]:
        acc += corer["outT"].astype(np.float32)
    out = np.ascontiguousarray(acc.T).reshape(B, T, C)
    # v-bias and c_proj bias fold into the host-side reduction epilogue:
    # softmax rows sum to 1, so b_v contributes b_v @ W_proj to every row.
    b_attn = np.asarray(b_attn, np.float32)
    out += np.asarray(b_proj, np.float32) + b_attn[2 * C:] @ np.asarray(
        W_proj, np.float32)
    return out


# revision 9
# speedup vs baseline: 1.0421x; 1.0016x over previous
"""Causal self-attention (B=2, T=2048, C=1024, H=16) on 8 Trainium2 cores.

Sharding: tensor-parallel over heads (2 heads/core). Each core computes
QKV projection for its heads, causal attention, and a partial c_proj
output; partials are summed on the host. The v-projection bias and
b_proj fold into the host reduction (softmax weights sum to 1, so the
v-bias contributes the constant vector b_v @ W_proj to every row).

All matmuls run in bf16 (1 PE cycle/row at any free size under the
cost model; rel err budget is 2e-2 and bf16 lands ~1e-3 end to end).

Per-core dataflow, everything K-major so no PE transposes at all:
  xT [C, B*T] bf16 (host pre-transposes x)
  qT/kT [128, B*T] = Wqk^T @ x + b            (PE; epilogue adds bias)
  v     [r 128, 65]  = x^T-stationary matmul  (PE; direct [r, hs] layout,
                                               ones col for softmax sums)
  S^T pair [k 128, 2*512] = K @ Q^T           (PE; causal tiles only)
  E^T = exp(S^T/8) over the flat [128, <=1024] span  (ACT, 2 tiles/inst)
  diag 128x128 blocks masked post-exp         (Pool, 0/1 trimask)
  y'^T [65, 512] += v_aug^T @ E^T             (PE; row 64 = softmax sums)
  rec = 1/y'[64] (DVE, PSUM direct); bcast via rank-1 PE matmul
  ynT = y'[0:64] * bcast                      (DVE)
  partial^T [c 128, 512] = Wp_local^T @ ynT   (PE) -> bf16 -> DRAM

The PE executes in order, so emission order = PE schedule: background
work (next batch's QKV/v projections, c_proj row tiles) is drip-fed
into the attention kc loop one thunk at a time to cover the exp
latency (ACT is slightly slower per tile pair than PE).
"""

import numpy as np
import ml_dtypes

import concourse.bass as bass
import concourse.tile as tile
from concourse import bacc, mybir
from concourse.bass_utils import run_bass_kernel_spmd

F32 = mybir.dt.float32
F32R = mybir.dt.float32r
BF16 = mybir.dt.bfloat16

B, T, C, H = 2, 2048, 1024, 16
HS = C // H            # 64 head dim
NCORES = 8
HL = H // NCORES       # 2 local heads
LC = HL * HS           # 128 local q/k/v cols
R = B * T              # 4096 rows
KC = C // 128          # 8 contraction chunks for projections
QT = 512               # attention q tile
NQT = T // QT          # 4
KA = 128               # attention k chunk
RT = 512               # row tile for projections
NRT = R // RT          # 8
NCC = C // 128         # 8 c_proj output chunks
BF = ml_dtypes.bfloat16


def _flat(t, lo, hi):
    """Contiguous free-dim span [lo, hi) of a tile viewed as [part, hi-lo]."""
    return bass.AP(tensor=t.tensor, offset=t.offset + lo,
                   ap=[list(t.ap[0]), [1, hi - lo]])


def build_program():
    nc = bacc.Bacc("TRN2", target_bir_lowering=False, debug=False,
                   num_devices=NCORES)

    xT = nc.dram_tensor("xT", [C, R], BF16, kind="ExternalInput").ap()
    wqk = nc.dram_tensor("wqk", [C, 2 * LC], BF16, kind="ExternalInput").ap()
    wv = nc.dram_tensor("wv", [C, LC], BF16, kind="ExternalInput").ap()
    bqk = nc.dram_tensor("bqk", [2 * LC], F32, kind="ExternalInput").ap()
    wp = nc.dram_tensor("wp", [LC, C], BF16, kind="ExternalInput").ap()
    trimask = nc.dram_tensor("trimask", [KA, KA], BF16,
                             kind="ExternalInput").ap()
    outT = nc.dram_tensor("outT", [C, R], BF16, kind="ExternalOutput").ap()

    with tile.TileContext(nc) as tc:
        with (
            tc.tile_pool(name="consts", bufs=1) as consts,
            tc.tile_pool(name="weights", bufs=1) as weights,
            tc.tile_pool(name="qkvT", bufs=1) as qkvT_pool,
            tc.tile_pool(name="xs", bufs=NRT) as xs_pool,
            tc.tile_pool(name="vh", bufs=2 * B * T // KA) as vh_pool,
            tc.tile_pool(name="et", bufs=4) as et_pool,
            tc.tile_pool(name="rec", bufs=2) as rec_pool,
            tc.tile_pool(name="osb", bufs=4) as osb_pool,
            tc.tile_pool(name="dscr", bufs=4, space="DRAM") as dscr_pool,
            tc.tile_pool(name="mm512", bufs=2, space="PSUM") as mm512_pool,
            tc.tile_pool(name="ytps", bufs=2, space="PSUM") as ytps_pool,
            tc.tile_pool(name="smps", bufs=2, space="PSUM") as smps_pool,
        ):
            # ---- constants ----
            ones64_f = consts.tile([1, HS], F32)
            nc.vector.memset(ones64_f, 1.0)
            ones64 = consts.tile([1, HS], F32R)
            nc.vector.tensor_copy(ones64, ones64_f)
            tri_sb = consts.tile([KA, KA], BF16)
            bqk_sb = consts.tile([128, 2], F32)

            wqk_sb = weights.tile([128, KC, 2 * LC], BF16)
            wv_sb = weights.tile([128, KC, LC], BF16)
            wp_sb = weights.tile([LC, C], BF16)

            wqk_r = wqk.rearrange("(kc p) n -> p kc n", p=128)
            wv_r = wv.rearrange("(kc p) n -> p kc n", p=128)

            # minimal first loads so PE starts fast; weights on SP queue,
            # x on ACT queue (all x is prefetched up front: no DMA waits
            # inside the attention phase)
            nc.sync.dma_start(out=wqk_sb[:, 0:1], in_=wqk_r[:, 0:1])
            nc.sync.dma_start(
                out=bqk_sb, in_=bqk.rearrange("(j p) -> p j", p=128))
            x_tiles = []
            for rt in range(NRT):
                x_sb = xs_pool.tile([128, KC, RT], BF16, tag="xs",
                                    name=f"x_sb{rt}")
                x_tiles.append(x_sb)
            x_r0 = xT[:, 0:RT].rearrange("(kc p) r -> p kc r", p=128)
            nc.scalar.dma_start(out=x_tiles[0][:, 0:1], in_=x_r0[:, 0:1])
            nc.scalar.dma_start(out=x_tiles[0][:, 1:4], in_=x_r0[:, 1:4])
            nc.sync.dma_start(out=wqk_sb[:, 1:4], in_=wqk_r[:, 1:4])
            nc.scalar.dma_start(out=x_tiles[0][:, 4:KC], in_=x_r0[:, 4:KC])
            nc.sync.dma_start(out=wqk_sb[:, 4:KC], in_=wqk_r[:, 4:KC])
            nc.sync.dma_start(out=wv_sb, in_=wv_r)
            nc.sync.dma_start(out=tri_sb, in_=trimask)
            nc.sync.dma_start(out=wp_sb, in_=wp)
            for rt in range(1, NRT):
                x_r = xT[:, rt * RT:(rt + 1) * RT].rearrange(
                    "(kc p) r -> p kc r", p=128)
                nc.scalar.dma_start(out=x_tiles[rt][:, 0:KC // 2],
                                    in_=x_r[:, 0:KC // 2])
                nc.scalar.dma_start(out=x_tiles[rt][:, KC // 2:],
                                    in_=x_r[:, KC // 2:])

            qT_s = qkvT_pool.tile([LC, R], BF16, tag="qT")
            kT_s = qkvT_pool.tile([LC, R], BF16, tag="kT")
            ynT_s = qkvT_pool.tile([LC, R], BF16, tag="ynT")

            vh_tiles = {}   # (b, chunk) -> [128, 130] tile (65 per head)
            epi_rr = [0]    # epilogue engine round-robin state

            def psum_to_sbuf(dst, src, bias=None):
                """PSUM->SBUF epilogue, mostly on DVE (ACT is exp-bound
                during attention; it takes every third copy only)."""
                i = epi_rr[0]
                epi_rr[0] += 1
                on_act = (i % 3 == 2)
                if bias is not None:
                    if on_act:
                        nc.scalar.activation(
                            dst, src, mybir.ActivationFunctionType.Identity,
                            bias=bias)
                    else:
                        nc.vector.tensor_scalar_add(dst, src, bias)
                    return
                if on_act:
                    nc.scalar.activation(
                        dst, src, mybir.ActivationFunctionType.Identity)
                else:
                    nc.vector.tensor_copy(dst, src)

            def qk_mm(rt, col, lohi, ps):
                dst = qT_s if col == 0 else kT_s
                for kc in range(*lohi):
                    nc.tensor.matmul(
                        ps,
                        wqk_sb[:, kc, col * LC:(col + 1) * LC],
                        x_tiles[rt][:, kc, :],
                        start=(kc == 0),
                        stop=(kc == KC - 1),
                    )
                if lohi[1] == KC:
                    psum_to_sbuf(dst[:, rt * RT:(rt + 1) * RT], ps,
                                 bias=bqk_sb[:, col:col + 1])

            def v_mm(rt, j, ps):
                """v for row chunk (rt*4+j) directly in [r, hs] layout."""
                for kc in range(KC):
                    nc.tensor.matmul(
                        ps[:, j * KA:(j + 1) * KA],
                        x_tiles[rt][:, kc, j * KA:(j + 1) * KA],
                        wv_sb[:, kc, :],
                        start=(kc == 0),
                        stop=(kc == KC - 1),
                    )

            def v_epi(rt, j, ps):
                chunk = rt * (RT // KA) + j
                b = chunk // (T // KA)
                ch = chunk % (T // KA)
                vh = vh_pool.tile([128, 2 * (HS + 1)], BF16, tag="vh",
                                  name=f"vh_b{b}c{ch}")
                src = ps[:, j * KA:(j + 1) * KA]
                nc.vector.tensor_copy(
                    bass.AP(tensor=vh.tensor, offset=vh.offset,
                            ap=[list(vh.ap[0]), [HS + 1, 2], [1, HS]]),
                    bass.AP(tensor=src.tensor, offset=src.offset,
                            ap=[list(src.ap[0]), [HS, 2], [1, HS]]),
                )
                nc.gpsimd.memset(
                    bass.AP(tensor=vh.tensor, offset=vh.offset + HS,
                            ap=[list(vh.ap[0]), [HS + 1, 2], [1, 1]]),
                    1.0)
                vh_tiles[(b, ch)] = vh

            def qkv_rt_thunks(rt):
                """Thunk list for one 512-row QKV tile (~2.5us of PE)."""
                ps_qk = [None, None]
                ps_v = [None]

                def qk_a(col):
                    def f():
                        ps_qk[col] = mm512_pool.tile(
                            [128, RT], F32, tag="mm512",
                            name=f"qkps{rt}c{col}")
                        qk_mm(rt, col, (0, KC // 2), ps_qk[col])
                    return f

                def qk_b(col):
                    return lambda: qk_mm(rt, col, (KC // 2, KC), ps_qk[col])

                def v_a(j):
                    def f():
                        if j == 0:
                            ps_v[0] = mm512_pool.tile(
                                [128, RT], F32, tag="mm512",
                                name=f"vps{rt}")
                        v_mm(rt, j, ps_v[0])
                    return f

                def v_b(j):
                    return lambda: v_epi(rt, j, ps_v[0])

                out = [qk_a(0), qk_b(0), qk_a(1), qk_b(1)]
                for j in range(RT // KA):
                    out.append(v_a(j))
                for j in range(RT // KA):
                    out.append(v_b(j))
                return out

            o2_state = {}

            def proj_thunk(rt, cc):
                """One c_proj output chunk: matmul + epilogue (+store)."""
                def f():
                    pp = mm512_pool.tile([128, RT], F32, tag="mm512",
                                         name=f"pp{rt}c{cc}")
                    nc.tensor.matmul(
                        pp,
                        wp_sb[:, cc * 128:(cc + 1) * 128],
                        ynT_s[:, rt * RT:(rt + 1) * RT],
                        start=True,
                        stop=True,
                    )
                    if cc % 2 == 0:
                        o2_state[rt] = osb_pool.tile(
                            [128, 2, RT], BF16, tag="osb",
                            name=f"o2_{rt}_{cc}")
                    o2 = o2_state[rt]
                    psum_to_sbuf(o2[:, cc % 2], pp)
                    if cc % 2 == 1:
                        dst = outT[(cc - 1) * 128:(cc + 1) * 128,
                                   rt * RT:(rt + 1) * RT]
                        nc.sync.dma_start(
                            out=dst.rearrange("(g p) r -> p g r", p=128),
                            in_=o2)
                return f

            bg = []

            def bg_step(n=1):
                for _ in range(n):
                    if bg:
                        bg.pop(0)()

            def attn_qt(b, h, qt, tail=False):
                base = b * T
                hsl = slice(h * HS, (h + 1) * HS)
                qcols = slice(base + qt * QT, base + (qt + 1) * QT)
                nka = (qt + 1) * (QT // KA)
                pairs = [(kc, kc + 1) for kc in range(0, nka, 2)]
                pend = None

                yp = ytps_pool.tile([HS + 1, QT], F32, tag="yt",
                                    name=f"yt_b{b}h{h}q{qt}")

                def av_pair(info):
                    sps2, et2, p2 = info
                    for i, kc in enumerate(p2):
                        diag = (kc * KA // QT == qt)
                        off = kc * KA - qt * QT if diag else 0
                        nc.tensor.matmul(
                            yp[:, off:QT],
                            vh_tiles[(b, kc)][:, h * (HS + 1):
                                              (h + 1) * (HS + 1)],
                            et2[:, i, off:QT],
                            start=(kc == 0),
                            stop=(kc == nka - 1),
                        )

                for p2 in pairs:
                    sps2 = smps_pool.tile(
                        [128, 2, QT], F32, tag="sm",
                        name=f"sps_b{b}h{h}q{qt}k{p2[0]}")
                    for i, kc in enumerate(p2):
                        nc.tensor.matmul(
                            sps2[:, i],
                            kT_s[hsl, base + kc * KA:base + (kc + 1) * KA],
                            qT_s[hsl, qcols],
                            start=True,
                            stop=True,
                        )
                    et2 = et_pool.tile([128, 2, QT], BF16, tag="et",
                                       name=f"et_b{b}h{h}q{qt}k{p2[0]}")
                    offs = [kc * KA - qt * QT if kc * KA // QT == qt else 0
                            for kc in p2]
                    scale = 1.0 / np.sqrt(HS).item()
                    if offs[1] <= KA:
                        # one flat span; <=128 wasted rows beat a 2nd init
                        nc.scalar.activation(
                            _flat(et2, offs[0], 2 * QT),
                            _flat(sps2, offs[0], 2 * QT),
                            mybir.ActivationFunctionType.Exp, scale=scale)
                    else:
                        for i in range(2):
                            nc.scalar.activation(
                                _flat(et2, i * QT + offs[i], (i + 1) * QT),
                                _flat(sps2, i * QT + offs[i], (i + 1) * QT),
                                mybir.ActivationFunctionType.Exp,
                                scale=scale)
                    for i, kc in enumerate(p2):
                        if kc * KA // QT == qt:
                            o = kc * KA - qt * QT
                            nc.gpsimd.tensor_mul(
                                et2[:, i, o:o + KA], et2[:, i, o:o + KA],
                                tri_sb)
                    if pend is not None:
                        av_pair(pend)
                    pend = (sps2, et2, p2)
                    bg_step(2 if len(bg) > 24 else 1)
                av_pair(pend)

                # softmax normalize. Default: DMA-bounce broadcast of the
                # reciprocal row (no PE/ACT cost, ~3.5us latency hidden by
                # the next qt). Tail: rank-1 PE broadcast + SBUF bounce of
                # y, the shortest-latency chain.
                rec = rec_pool.tile([1, QT], F32R, tag="rec",
                                    name=f"rec_b{b}h{h}q{qt}")
                with nc.allow_low_precision(
                        reason="f32r reciprocal: ~1e-4 rel err ok"):
                    nc.vector.reciprocal(rec, yp[HS:HS + 1, :])
                if tail:
                    yc = rec_pool.tile([HS, QT], F32, tag="yc",
                                       name=f"yc_b{b}h{h}q{qt}")
                    nc.scalar.activation(
                        yc, yp[0:HS, :],
                        mybir.ActivationFunctionType.Identity)
                    bcp = mm512_pool.tile([HS, QT], F32, tag="mm512",
                                          name=f"bcp_b{b}h{h}q{qt}")
                    nc.tensor.matmul(bcp, ones64, rec, start=True, stop=True)
                    nc.vector.tensor_mul(ynT_s[hsl, qcols], yc, bcp)
                else:
                    recd = dscr_pool.tile([1, QT], F32R, tag="recd",
                                          name=f"recd_b{b}h{h}q{qt}")
                    nc.sync.dma_start(out=recd, in_=rec)
                    bcs = rec_pool.tile([HS, QT], F32R, tag="bcs",
                                        name=f"bcs_b{b}h{h}q{qt}")
                    rec_bcast = bass.AP(
                        tensor=recd.tensor, offset=recd.offset,
                        ap=[[0, HS]] + [list(d) for d in recd.ap[1:]])
                    nc.sync.dma_start(out=bcs, in_=rec_bcast)
                    nc.vector.tensor_mul(ynT_s[hsl, qcols], yp[0:HS, :], bcs)
                bg_step(2)

            # ---- phase A: QKV+v for batch 0, head-0 attention interleaved
            # per row tile so PE always has non-DMA-gated work; batch-1 QKV
            # queued as background filler ----
            for rt in range(NRT // 2):
                for th in qkv_rt_thunks(rt):
                    th()
                bg.extend(qkv_rt_thunks(NRT // 2 + rt))
                attn_qt(0, 0, rt)

            # ---- batch 0 head 1 (bg: batch-1 QKV; proj rows as they land)
            for qt in range(NQT):
                attn_qt(0, 1, qt)
                for cc in range(NCC):
                    bg.append(proj_thunk(qt, cc))

            # ---- batch 1 (bg: remaining proj rows) ----
            for qt in range(NQT):
                attn_qt(1, 0, qt)
            for qt in range(NQT):
                attn_qt(1, 1, qt, tail=(qt == NQT - 1))
                for cc in range(NCC):
                    bg.append(proj_thunk(NRT // 2 + qt, cc))
            while bg:
                bg_step(1)

    nc.compile()
    return nc


_NC = None


def _get_nc():
    global _NC
    if _NC is None:
        _NC = build_program()
    return _NC


def make_in_maps(x, W_attn, b_attn, W_proj, b_proj):
    x = np.asarray(x, np.float32)
    W_attn = np.asarray(W_attn, np.float32)
    b_attn = np.asarray(b_attn, np.float32)
    W_proj = np.asarray(W_proj, np.float32)

    xT = np.ascontiguousarray(x.reshape(R, C).T).astype(BF)
    tri = np.triu(np.ones((KA, KA), np.float32)).astype(BF)

    in_maps = []
    for core in range(NCORES):
        g0 = core * LC
        cols = slice(g0, g0 + LC)
        wqk_l = np.concatenate(
            [W_attn[:, 0:C][:, cols], W_attn[:, C:2 * C][:, cols]], axis=1)
        bqk_l = np.concatenate(
            [b_attn[0:C][cols], b_attn[C:2 * C][cols]])
        in_maps.append({
            "xT": xT,
            "wqk": np.ascontiguousarray(wqk_l).astype(BF),
            "wv": np.ascontiguousarray(
                W_attn[:, 2 * C:3 * C][:, cols]).astype(BF),
            "bqk": np.ascontiguousarray(bqk_l),
            "wp": np.ascontiguousarray(W_proj[cols, :]).astype(BF),
            "trimask": tri,
        })
    return in_maps


def kernel(x, W_attn, b_attn, W_proj, b_proj):
    nc = _get_nc()
    in_maps = make_in_maps(x, W_attn, b_attn, W_proj, b_proj)
    res = run_bass_kernel_spmd(nc, in_maps, list(range(NCORES)))
    acc = res.results[0]["outT"].astype(np.float32)
    for corer in res.results[1:]:
        acc += corer["outT"].astype(np.float32)
    out = np.ascontiguousarray(acc.T).reshape(B, T, C)
    # v-bias and c_proj bias fold into the host-side reduction epilogue:
    # softmax rows sum to 1, so b_v contributes b_v @ W_proj to every row.
    b_attn = np.asarray(b_attn, np.float32)
    out += np.asarray(b_proj, np.float32) + b_attn[2 * C:] @ np.asarray(
        W_proj, np.float32)
    return out
